# revision 1
# baseline (speedup 1.0000x reference)
"""Trainium2 Bass kernel for nn_CHUNKER (ragged span scorer).

Strategy: data-parallel over the 40770 spans across 8 NeuronCores (padded to
8 x 5120). Each core redundantly computes the token-embedding prefix sum
(csum, [2049, 512] fp32) and writes it to DRAM: the word-embedding half via
indirect-DMA gathers + triangular matmuls, the pos-tag half gather-free via
an exact one-hot / cumulative-count matmul factorization
(tri @ onehot(ptag) @ We_pos). Span csum rows are then gathered per span
with indirect DMA, span means formed (fp32 subtract, 1/len scale), and the
MLP stack runs in feature-major layout with fp16 matmuls (full PE rate with
fast weight load; ~5e-4 relative error). Sinusoidal span-geometry features
are computed on-chip with an exact round-to-nearest range reduction feeding
the ScalarE Sin LUT ([-pi, pi] domain).
"""

import os
import numpy as np
from contextlib import ExitStack

import concourse.bass as bass
import concourse.tile as tile
from concourse import bacc, mybir
from concourse.bass_utils import run_bass_kernel_spmd

P = 128
N_TOKENS = 2048
VOCAB = 50000
N_TAGS = 53
WDIM = 256
EMB = 2 * WDIM          # 512
HDIM = 512
POS_DIM = 32
FEAT = 4 * POS_DIM      # 128
N_CORES = 8
SH = 5120               # spans per core (padded); 8*5120 = 40960 >= 40770
NB = SH // 512          # 10 blocks of 512 spans
NTB = N_TOKENS // P     # 16 token blocks

f32 = mybir.dt.float32
f32r = mybir.dt.float32r
f16 = mybir.dt.float16
i32 = mybir.dt.int32
AF = mybir.ActivationFunctionType
MAGIC = 12582912.0      # 1.5 * 2**23: (x + MAGIC) - MAGIC == round-to-nearest(x)
TWO_PI = float(np.float32(2.0 * np.pi))

_CACHE = {}
LAST_RESULT = None


def _consts():
    triu = np.triu(np.ones((P, P), np.float32))           # [k,m]=1 if k<=m
    tril = np.triu(np.ones((P, P), np.float32))           # [k,t]=1 if k<=t (cumcount rhs)
    scan = np.triu(np.ones((NTB, NTB), np.float32), 1)    # strict upper
    ident = np.eye(P, dtype=np.float32)
    iota_tags = np.tile(np.arange(N_TAGS, dtype=np.float32)[None, :], (P, 1))
    # B3 [3, 128]: u[m, s] = B3[0,m]*start[s] + B3[1,m]*len[s] + B3[2,m]
    # u = angle in turns: pos_g * freq / 2pi + phase (0 or 0.25)
    # pos kinds per group g: start(1,0), end(1,1), len(0,1), mid(1,0.5)
    freq = np.exp(-np.log(10000.0) * (2.0 * np.arange(16) / POS_DIM)).astype(np.float64)
    c01 = [(1.0, 0.0), (1.0, 1.0), (0.0, 1.0), (1.0, 0.5)]
    b4 = np.zeros((4, P), np.float64)
    for m in range(P):
        g, j = m // 32, m % 32
        jj = j if j < 16 else j - 16
        fu = freq[jj] / (2.0 * np.pi)
        b4[0, m] = fu * c01[g][0]   # coeff on s_hi
        b4[1, m] = fu * c01[g][1]   # coeff on len
        b4[2, m] = fu * c01[g][0]   # coeff on s_lo
        b4[3, m] = 0.0 if j < 16 else 0.25
    b4h = b4.astype(np.float16)
    b4l = (b4 - b4h.astype(np.float64)).astype(np.float16)
    return triu, tril, scan, ident, iota_tags, b4h, b4l


def _build_nc():
    nc = bacc.Bacc("TRN2", target_bir_lowering=False, debug=False,
                   num_devices=N_CORES)

    def inp(name, shape, dt):
        return nc.dram_tensor(name, shape, dt, kind="ExternalInput").ap()

    sent = inp("sentence", [N_TOKENS], i32)
    ptag = inp("pos_tags", [N_TOKENS], i32)
    sst = inp("sst", [SH], i32)
    sln = inp("sln", [SH], i32)
    we_w = inp("We_wrd", [VOCAB, WDIM], f32)
    we_p = inp("We_pos", [N_TAGS, WDIM], f32)
    w0 = inp("dan_w0", [EMB, HDIM], f32)
    b0 = inp("dan_b0", [HDIM], f32)
    w1 = inp("dan_w1", [HDIM, HDIM], f32)
    b1 = inp("dan_b1", [HDIM], f32)
    ws0 = inp("ws_w0", [HDIM + FEAT, HDIM], f32)
    bs0 = inp("ws_b0", [HDIM], f32)
    ws1 = inp("ws_w1", [HDIM, 1], f32)
    bs1 = inp("ws_b1", [1], f32)

    scores = nc.dram_tensor("scores", [SH], f32, kind="ExternalOutput").ap()
    KDBG = os.environ.get("KDBG", "0") == "1"
    if KDBG:
        smdbg = nc.dram_tensor("smdbg", [NB * 4 * P, 512], f32,
                               kind="ExternalOutput").ap()
    csum_d = nc.dram_tensor("csum_d", [N_TOKENS + 1, EMB], f32,
                            kind="ExternalOutput" if KDBG else "Internal").ap()
    carry_hd = nc.dram_tensor("carry_hd", [NTB, EMB], f16, kind="Internal").ap()
    carry_ld = nc.dram_tensor("carry_ld", [NTB, EMB], f16, kind="Internal").ap()
    totals_d = nc.dram_tensor("totals_d", [NTB, EMB], f32, kind="Internal").ap()
    warm_d = nc.dram_tensor("warm_d", [P, P], f32, kind="Internal").ap()

    triu_np, tril_np, scan_np, ident_np, iota_np, b4h_np, b4l_np = _consts()
    triu_c = nc.inline_tensor(triu_np.astype(np.float16), "triu_c").ap()
    tril_c = nc.inline_tensor(tril_np.astype(np.float16), "tril_c").ap()
    scan_c = nc.inline_tensor(scan_np.astype(np.float16), "scan_c").ap()
    ident_c = nc.inline_tensor(ident_np, "ident_c").ap()
    iota_c = nc.inline_tensor(iota_np, "iota_c").ap()
    b4h_c = nc.inline_tensor(b4h_np, "b4h_c").ap()
    b4l_c = nc.inline_tensor(b4l_np, "b4l_c").ap()
    ones2_c = nc.inline_tensor(np.ones((2, P), np.float16), "ones2_c").ap()
    ones512h_c = nc.inline_tensor(np.ones((1, 512), np.float16), "ones512h_c").ap()

    with tile.TileContext(nc) as tc, ExitStack() as ctx:
        wp = ctx.enter_context(tc.tile_pool(name="wp", bufs=1))
        stg = ctx.enter_context(tc.tile_pool(name="stg", bufs=2))

        # ---- index tiles + word-embedding gathers FIRST (ahead of the big
        # weight DMAs) so GpSimd descriptor-gen starts immediately ----
        sent_i = wp.tile([P, NTB], i32)
        nc.sync.dma_start(sent_i[:], sent.rearrange("(b p) -> p b", p=P))
        ptag_i = wp.tile([P, NTB], i32)
        nc.sync.dma_start(ptag_i[:], ptag.rearrange("(b p) -> p b", p=P))
        sst_t = wp.tile([P, 40], i32)
        nc.sync.dma_start(sst_t[:], sst.rearrange("(j p) -> p j", p=P))
        sln_t = wp.tile([P, 40], i32)
        nc.sync.dma_start(sln_t[:], sln.rearrange("(j p) -> p j", p=P))

        ap_ctx = ExitStack()
        ap_sb = ap_ctx.enter_context(tc.tile_pool(name="ap_sb", bufs=1))
        ap_ps = ap_ctx.enter_context(tc.tile_pool(name="ap_ps", bufs=1, space="PSUM"))
        ap_out = ap_ctx.enter_context(tc.tile_pool(name="ap_out", bufs=3))

        # wrd gathers: emb_w[:, blk*WDIM:...] = We_wrd[sentence[blk]]
        emb_w = ap_sb.tile([P, NTB * WDIM], f16)
        for blk in range(NTB):
            nc.gpsimd.indirect_dma_start(
                out=emb_w[:, blk * WDIM:(blk + 1) * WDIM],
                out_offset=None, in_=we_w[:],
                in_offset=bass.IndirectOffsetOnAxis(ap=sent_i[:, blk:blk + 1], axis=0))

        # ---- small constants ----
        triu_t = wp.tile([P, P], f16)
        nc.sync.dma_start(triu_t[:], triu_c[:])
        tril_h = wp.tile([P, P], f16)
        nc.sync.dma_start(tril_h[:], tril_c[:])
        scan_t = wp.tile([NTB, NTB], f16)
        nc.sync.dma_start(scan_t[:], scan_c[:])
        b4h_t = wp.tile([4, P], f16)
        nc.sync.dma_start(b4h_t[:], b4h_c[:])
        b4l_t = wp.tile([4, P], f16)
        nc.sync.dma_start(b4l_t[:], b4l_c[:])
        iota_t = wp.tile([P, N_TAGS], f32)
        nc.sync.dma_start(iota_t[:], iota_c[:])
        wep_t = wp.tile([N_TAGS, WDIM], f32)
        nc.sync.dma_start(wep_t[:], we_p[:])
        wep_hi = wp.tile([N_TAGS, WDIM], f16)
        nc.vector.tensor_copy(wep_hi[:], wep_t[:])
        idstg = stg.tile([P, P], f32, tag="idstg")
        nc.sync.dma_start(idstg[:], ident_c[:])
        ident_h = wp.tile([P, P], f16)
        nc.vector.tensor_copy(ident_h[:], idstg[:])
        ones2_h = wp.tile([2, P], f16)
        nc.sync.dma_start(ones2_h[:], ones2_c[:])

        # derived span indices
        send_t = wp.tile([P, 40], i32)
        nc.vector.tensor_tensor(out=send_t[:], in0=sst_t[:], in1=sln_t[:],
                                op=mybir.AluOpType.add)
        nc.vector.tensor_scalar_min(send_t[:], send_t[:], N_TOKENS)
        lenf_t = wp.tile([P, 40], f32)
        nc.vector.tensor_copy(lenf_t[:], sln_t[:])
        recip_t = wp.tile([P, 40], f32)
        nc.vector.reciprocal(recip_t[:], lenf_t[:])
        ptag_f = wp.tile([P, NTB], f32)
        nc.vector.tensor_copy(ptag_f[:], ptag_i[:])

        # ---- stage A: per-block inclusive csum -> csum_d ----
        # pos half gather-free: csum_pos = cumcount(onehot(ptag)) @ We_pos
        emb_big = ap_sb.tile([P, NTB * EMB], f32)  # becomes block csum
        for blk in range(NTB):
            oh = ap_sb.tile([P, N_TAGS], f16, tag="oh", name=f"oh{blk}", bufs=2)
            nc.vector.tensor_scalar(out=oh[:], in0=iota_t[:],
                                    scalar1=ptag_f[:, blk:blk + 1], scalar2=None,
                                    op0=mybir.AluOpType.is_equal)
            ccT_ps = ap_ps.tile([N_TAGS, P], f32, tag="ccT", name=f"ccT{blk}", bufs=2)
            nc.tensor.matmul(ccT_ps[:], oh[:], tril_h[:], start=True, stop=True)
            ccT = ap_sb.tile([N_TAGS, P], f16, tag="ccTs", name=f"ccTs{blk}", bufs=3)
            nc.vector.tensor_copy(ccT[:], ccT_ps[:])  # exact small ints
            tp = ap_ps.tile([P, EMB], f32, tag="tri_ps", name=f"tp{blk}", bufs=2)
            nc.tensor.matmul(tp[:, 0:WDIM], ccT[:], wep_hi[:],
                             start=True, stop=True)
            nc.tensor.matmul(tp[:, WDIM:EMB], triu_t[:],
                             emb_w[:, blk * WDIM:(blk + 1) * WDIM],
                             start=True, stop=True)
            nc.vector.tensor_copy(emb_big[:, blk * EMB:(blk + 1) * EMB], tp[:])
        bcsum = emb_big

        nc.sync.dma_start(totals_d.rearrange("b f -> (b f)")[None, :],
                          bcsum[P - 1:P, :])
        totals = ap_sb.tile([NTB, EMB], f32)
        nc.sync.dma_start(totals[:], totals_d[:])
        tot_hi = ap_sb.tile([NTB, EMB], f16)
        nc.vector.tensor_copy(tot_hi[:], totals[:])
        tot_lo = ap_sb.tile([NTB, EMB], f16)
        nc.vector.tensor_tensor(out=tot_lo[:], in0=totals[:], in1=tot_hi[:],
                                op=mybir.AluOpType.subtract)
        carry_ps = ap_ps.tile([NTB, EMB], f32, tag="carry_ps", bufs=1)
        nc.tensor.matmul(carry_ps[:], scan_t[:], tot_hi[:], start=True, stop=False)
        nc.tensor.matmul(carry_ps[:], scan_t[:], tot_lo[:], start=False, stop=True)
        # hi/lo fp16 split keeps the K=1 broadcast matmuls fast AND exact
        carry_hi = ap_sb.tile([NTB, EMB], f16)
        nc.vector.tensor_copy(carry_hi[:], carry_ps[:])
        carry_lo = ap_sb.tile([NTB, EMB], f16)
        nc.vector.tensor_tensor(out=carry_lo[:], in0=carry_ps[:],
                                in1=carry_hi[:], op=mybir.AluOpType.subtract)
        nc.sync.dma_start(carry_hd[:], carry_hi[:])
        nc.scalar.dma_start(carry_ld[:], carry_lo[:])
        carry_2 = ap_sb.tile([2, NTB * EMB], f16)
        nc.sync.dma_start(carry_2[0:1, :], carry_hd.rearrange("b f -> (b f)")[None, :])
        nc.scalar.dma_start(carry_2[1:2, :], carry_ld.rearrange("b f -> (b f)")[None, :])

        zrow = ap_sb.tile([1, EMB], f32)
        nc.vector.memset(zrow[:], 0.0)
        nc.sync.dma_start(csum_d[0:1, :], zrow[:])
        for blk in range(NTB):
            bc_ps = ap_ps.tile([P, EMB], f32, tag="bc_ps", name=f"bc{blk}", bufs=3)
            nc.tensor.matmul(bc_ps[:], ones2_h[:],
                             carry_2[:, blk * EMB:(blk + 1) * EMB],
                             start=True, stop=True)
            co = ap_out.tile([P, EMB], f32, tag="co", name=f"co{blk}", bufs=5)
            nc.vector.tensor_tensor(
                out=co[:], in0=bcsum[:, blk * EMB:(blk + 1) * EMB],
                in1=bc_ps[:], op=mybir.AluOpType.add)
            nc.sync.dma_start(csum_d[1 + blk * P:1 + (blk + 1) * P, :], co[:])
        ap_ctx.close()

        # ---- MLP weights / biases (fp16), loaded while stage A runs ----
        def load_wr(name, src_ap, n_fi):
            tiles = []
            for fi in range(n_fi):
                t = wp.tile([P, HDIM], f16, name=f"{name}{fi}", tag=f"{name}{fi}")
                s = stg.tile([P, HDIM], f32, tag="wstg", name=f"{name}s{fi}",
                             bufs=4)
                nc.scalar.dma_start(s[:], src_ap[fi * P:(fi + 1) * P, :])
                nc.vector.tensor_copy(t[:], s[:])
                tiles.append(t)
            return tiles

        w0_r = load_wr("w0r", w0, 4)
        w1_r = load_wr("w1r", w1, 4)
        ws0_r = load_wr("ws0r", ws0, 5)
        ws1_r = wp.tile([P, 4], f16)
        ws1_s = stg.tile([P, 4], f32, tag="ws1s")
        nc.scalar.dma_start(ws1_s[:], ws1.rearrange("(f p) o -> p (f o)", p=P))
        nc.vector.tensor_copy(ws1_r[:], ws1_s[:])

        def load_bias(name, src_ap):
            t = wp.tile([P, 4], f32, name=f"{name}_t", tag=f"{name}_t")
            nc.scalar.dma_start(t[:], src_ap.rearrange("(f p) -> p f", p=P))
            return t

        b0_t = load_bias("b0", b0)
        b1_t = load_bias("b1", b1)
        bs0_t = load_bias("bs0", bs0)
        bs1_t = wp.tile([1, 1], f32)
        nc.scalar.dma_start(bs1_t[:], bs1[None, :])

        # ---- PE warm-up chain across the csum-write -> first-gather gap ----
        with tc.tile_pool(name="warm_pool", bufs=1, space="PSUM") as warm_pool:
            warm_ps = warm_pool.tile([P, P], f32, tag="warm", bufs=1)
            for wi in range(190):
                nc.tensor.matmul(warm_ps[:], ident_h[:], ident_h[:],
                                 start=(wi == 0), stop=(wi == 189))
            warm_sb = stg.tile([P, P], f32, tag="warm_sb")
            nc.vector.tensor_copy(warm_sb[:], warm_ps[:])
            nc.sync.dma_start(warm_d[:], warm_sb[:])

        # ---- main span loop ----
        mp = ctx.enter_context(tc.tile_pool(name="mp", bufs=1))
        g_sb = ctx.enter_context(tc.tile_pool(name="g_sb", bufs=2))
        m_ps = ctx.enter_context(tc.tile_pool(name="m_ps", bufs=1, space="PSUM"))

        scores_sb = mp.tile([1, SH], f32)

        for b in range(NB):
            # --- feats: u (angle in turns) via K=3 fp32 matmul ---
            ps_i = g_sb.tile([2, 512], i32, tag="ps_i", name=f"ps_i{b}")
            nc.sync.dma_start(ps_i[0:1, :], sst[None, b * 512:(b + 1) * 512])
            nc.sync.dma_start(ps_i[1:2, :], sln[None, b * 512:(b + 1) * 512])
            pos2f = g_sb.tile([2, 512], f32, tag="pos2f", name=f"pos2f{b}")
            nc.vector.tensor_copy(pos2f[:], ps_i[:])
            pos4 = g_sb.tile([4, 512], f16, tag="pos4", name=f"pos4{b}")
            nc.vector.tensor_copy(pos4[0:2, :], pos2f[:])           # s_hi, len
            slo = g_sb.tile([1, 512], f16, tag="slo", name=f"slo{b}")
            nc.vector.tensor_tensor(out=slo[:], in0=pos2f[0:1, :],
                                    in1=pos4[0:1, :],
                                    op=mybir.AluOpType.subtract)    # s_lo exact
            nc.sync.dma_start(pos4[2:3, :], slo[:])
            nc.sync.dma_start(pos4[3:4, :], ones512h_c[:])

            u_ps = m_ps.tile([P, 512], f32, tag="u_ps", name=f"u_ps{b}", bufs=2)
            nc.tensor.matmul(u_ps[:], b4h_t[:], pos4[:], start=True, stop=False)
            nc.tensor.matmul(u_ps[:], b4l_t[:], pos4[:], start=False, stop=True)
            nf = g_sb.tile([P, 512], f32, tag="nf", name=f"nf{b}", bufs=1)
            nc.vector.tensor_scalar_add(nf[:], u_ps[:], MAGIC)
            nc.vector.tensor_scalar_sub(nf[:], nf[:], MAGIC)
            df = g_sb.tile([P, 512], f32, tag="df", name=f"df{b}", bufs=1)
            nc.vector.tensor_tensor(out=df[:], in0=u_ps[:], in1=nf[:],
                                    op=mybir.AluOpType.subtract)
            feat_t = g_sb.tile([P, 512], f16, tag="feat", name=f"feat{b}")
            nc.scalar.activation(feat_t[:], df[:], AF.Sin, scale=TWO_PI)

            # --- gather csum rows, span means, transpose to feature-major ---
            sm = []
            for j in range(4):
                c = 4 * b + j
                csE = g_sb.tile([P, EMB], f32, tag=f"csE{j}", name=f"csE{b}_{j}", bufs=4)
                nc.gpsimd.indirect_dma_start(
                    out=csE[:], out_offset=None, in_=csum_d[:],
                    in_offset=bass.IndirectOffsetOnAxis(ap=send_t[:, c:c + 1], axis=0))
                csS = g_sb.tile([P, EMB], f32, tag=f"csS{j}", name=f"csS{b}_{j}", bufs=4)
                nc.gpsimd.indirect_dma_start(
                    out=csS[:], out_offset=None, in_=csum_d[:],
                    in_offset=bass.IndirectOffsetOnAxis(ap=sst_t[:, c:c + 1], axis=0))
                sd = g_sb.tile([P, EMB], f16, tag=f"sd{j}", name=f"sd{b}_{j}")
                nc.vector.tensor_tensor(out=sd[:], in0=csE[:], in1=csS[:],
                                        op=mybir.AluOpType.subtract)
                smj = g_sb.tile([P, EMB], f16, tag=f"sm{j}", name=f"sm{b}_{j}")
                nc.vector.tensor_scalar(out=smj[:], in0=sd[:],
                                        scalar1=recip_t[:, c:c + 1], scalar2=None,
                                        op0=mybir.AluOpType.mult)
                sm.append(smj)

            a0 = []
            for f in range(4):
                aps = m_ps.tile([P, 512], f16, tag="a0ps", name=f"a0ps{b}_{f}", bufs=2)
                for j in range(4):
                    nc.tensor.matmul(
                        aps[:, j * P:(j + 1) * P],
                        sm[j][:, f * P:(f + 1) * P], ident_h[:],
                        is_transpose=True, start=True, stop=True)
                a0f = g_sb.tile([P, 512], f16, tag=f"a0_{f}", name=f"a0_{b}_{f}")
                nc.vector.tensor_copy(a0f[:], aps[:])
                if KDBG:
                    nc.gpsimd.dma_start(
                        out=smdbg[(b * 4 + f) * P:(b * 4 + f + 1) * P, :],
                        in_=a0f[:])
                a0.append(a0f)

            # --- MLP (fp16 weights/activations, fp32 psum) ---
            def layer(inputs, wts, bias_t, nfi, lname):
                outs = []
                for fo in range(4):
                    hp = m_ps.tile([P, 512], f32, tag="hps",
                                   name=f"{lname}{b}_{fo}", bufs=3)
                    for fi in range(nfi):
                        nc.tensor.matmul(hp[:],
                                         wts[fi][:, fo * P:(fo + 1) * P],
                                         inputs[fi][:],
                                         start=(fi == 0), stop=(fi == nfi - 1))
                    ot = g_sb.tile([P, 512], f16, tag=f"{lname}o{fo}",
                                   name=f"{lname}o{b}_{fo}")
                    nc.scalar.activation(ot[:], hp[:], AF.Relu,
                                         bias=bias_t[:, fo:fo + 1], scale=1.0)
                    outs.append(ot)
                return outs

            h1 = layer(a0, w0_r, b0_t, 4, "h1")
            h2 = layer(h1, w1_r, b1_t, 4, "h2")
            s3 = layer(h2 + [feat_t], ws0_r, bs0_t, 5, "s3")

            sc_ps = m_ps.tile([1, 512], f32, tag="sc_ps", name=f"sc_ps{b}", bufs=1)
            for fi in range(4):
                nc.tensor.matmul(sc_ps[:], ws1_r[:, fi:fi + 1], s3[fi][:],
                                 start=(fi == 0), stop=(fi == 3))
            nc.scalar.activation(scores_sb[0:1, b * 512:(b + 1) * 512], sc_ps[:],
                                 AF.Identity, bias=bs1_t[0:1, 0:1], scale=1.0)

        nc.sync.dma_start(scores[None, :], scores_sb[:])

    nc.compile()
    return nc


def kernel(**inputs):
    global LAST_RESULT
    if "nc" not in _CACHE:
        _CACHE["nc"] = _build_nc()
    nc = _CACHE["nc"]

    sst = np.zeros(N_CORES * SH, np.int32)
    sln = np.ones(N_CORES * SH, np.int32)
    n = inputs["spans_start"].shape[0]
    sst[:n] = np.asarray(inputs["spans_start"], np.int32)
    sln[:n] = np.asarray(inputs["spans_len"], np.int32)

    common = {
        "sentence": np.asarray(inputs["sentence"], np.int32),
        "pos_tags": np.asarray(inputs["pos_tags"], np.int32),
        "We_wrd": np.asarray(inputs["We_wrd"], np.float32),
        "We_pos": np.asarray(inputs["We_pos"], np.float32),
        "dan_w0": np.asarray(inputs["dan_w0"], np.float32),
        "dan_b0": np.asarray(inputs["dan_b0"], np.float32),
        "dan_w1": np.asarray(inputs["dan_w1"], np.float32),
        "dan_b1": np.asarray(inputs["dan_b1"], np.float32),
        "ws_w0": np.asarray(inputs["ws_w0"], np.float32),
        "ws_b0": np.asarray(inputs["ws_b0"], np.float32),
        "ws_w1": np.asarray(inputs["ws_w1"], np.float32),
        "ws_b1": np.asarray(inputs["ws_b1"], np.float32),
    }
    in_maps = []
    for c in range(N_CORES):
        m = dict(common)
        m["sst"] = sst[c * SH:(c + 1) * SH].copy()
        m["sln"] = sln[c * SH:(c + 1) * SH].copy()
        in_maps.append(m)

    res = run_bass_kernel_spmd(nc, in_maps, core_ids=list(range(N_CORES)))
    LAST_RESULT = res
    out = np.concatenate([res.results[c]["scores"] for c in range(N_CORES)])
    return out[:n]



# revision 7
# speedup vs baseline: 1.9546x; 1.9546x over previous
"""Trainium2 Bass kernel for nn_CHUNKER (ragged span scorer).

Fast path exploits the span structure: every span with start in [0, 2048)
and len in [1, 20] lies on a dense 2048 x 20 grid (= 8 x 5120 = exactly the
padded span count). Sharding the grid by 256-token start windows means each
core only touches ~276 consecutive tokens, so:
  - span means come from sliced subtracts of a local prefix sum (NO per-span
    csum gathers, no indirect DMA in the main loop),
  - the first MLP layer is folded into the prefix sum (Z = csum @ w0 over
    385 columns instead of 5120 spans),
  - everything stays feature-major (no PE transposes),
  - the sinusoidal geometry features reduce to one angle matmul at the
    prologue plus a per-len bias folded into the Sin activation.
Host side slices token windows per core and permutes the grid output back to
span order. Inputs whose spans fall off the grid (len<1 or len>20 etc.) fall
back to the original gather-based kernel, which is correct for any spans.
"""

import numpy as np
from contextlib import ExitStack

import concourse.bass as bass
import concourse.tile as tile
from concourse import bacc, mybir
from concourse.bass_utils import run_bass_kernel_spmd

P = 128
N_TOKENS = 2048
VOCAB = 50000
N_TAGS = 53
WDIM = 256
EMB = 2 * WDIM          # 512
HDIM = 512
POS_DIM = 32
FEAT = 4 * POS_DIM      # 128
N_CORES = 8
SH = 5120               # spans per core (grid: 20 lens x 256 starts)
NB = SH // 512          # 10 blocks of 512 spans (2 lens x 256 starts each)
NTB = N_TOKENS // P     # 16 token blocks (slow path)
WTOK = 384              # fast path: token window per core (3 blocks of 128)
NW = WTOK // P          # 3
NZ = 1 + WTOK           # prefix-sum columns (incl. leading zero)

f32 = mybir.dt.float32
f16 = mybir.dt.float16
i32 = mybir.dt.int32
AF = mybir.ActivationFunctionType
MAGIC = 12582912.0      # 1.5 * 2**23: (x + MAGIC) - MAGIC == round-to-nearest(x)
TWO_PI = float(np.float32(2.0 * np.pi))

_CACHE = {}
LAST_RESULT = None


# ---------------------------------------------------------------------------
# fast path
# ---------------------------------------------------------------------------

def _consts_fast():
    triu = np.triu(np.ones((P, P), np.float32))   # [k,t]=1 if k<=t
    onesk = np.ones((P, P), np.float32)
    # angle in turns: u[m] = A0[m]*start + A1[m]*len + A2[m]
    # feature groups g = m//32: start(1,0), end(1,1), len(0,1), mid(1,0.5);
    # j = m%32: j<16 sin(freq_j x), j>=16 cos = sin(. + 0.25 turns)
    freq = np.exp(-np.log(10000.0) * (2.0 * np.arange(16) / POS_DIM)).astype(np.float64)
    c01 = [(1.0, 0.0), (1.0, 1.0), (0.0, 1.0), (1.0, 0.5)]
    a0 = np.zeros(P, np.float64)
    a1 = np.zeros(P, np.float64)
    a2 = np.zeros(P, np.float64)
    for m in range(P):
        g, j = m // 32, m % 32
        jj = j if j < 16 else j - 16
        fu = freq[jj] / (2.0 * np.pi)
        a0[m] = fu * c01[g][0]
        a1[m] = fu * c01[g][1]
        a2[m] = 0.0 if j < 16 else 0.25
    a0h = a0.astype(np.float16)
    a0l = (a0 - a0h.astype(np.float64)).astype(np.float16)
    # per-len bias: bt (turns, added before rounding) and b2p = 2*pi*bt
    # (radians, folded into the Sin activation bias)
    bt = np.zeros((P, 20), np.float64)
    for l in range(1, 21):
        bt[:, l - 1] = a1 * l + a2
    b2p = 2.0 * np.pi * bt
    return (triu, onesk, a0h[None, :], a0l[None, :],
            bt.astype(np.float32), b2p.astype(np.float32))


def _build_nc_fast():
    nc = bacc.Bacc("TRN2", target_bir_lowering=False, debug=False,
                   num_devices=N_CORES)

    def inp(name, shape, dt):
        return nc.dram_tensor(name, shape, dt, kind="ExternalInput").ap()

    sent_w = inp("sent_w", [WTOK], i32)
    ptag_w = inp("ptag_w", [WTOK], i32)
    mask_w = inp("mask_w", [WTOK], f32)
    st2 = inp("starts2", [512], f32)
    we_w = inp("We_wrd", [VOCAB, WDIM], f32)
    we_p = inp("We_pos", [N_TAGS, WDIM], f32)
    w0 = inp("dan_w0", [EMB, HDIM], f32)
    b0 = inp("dan_b0", [HDIM], f32)
    w1 = inp("dan_w1", [HDIM, HDIM], f32)
    b1 = inp("dan_b1", [HDIM], f32)
    ws0 = inp("ws_w0", [HDIM + FEAT, HDIM], f32)
    bs0 = inp("ws_b0", [HDIM], f32)
    ws1 = inp("ws_w1", [HDIM, 1], f32)
    bs1 = inp("ws_b1", [1], f32)

    scores = nc.dram_tensor("scores", [SH], f32, kind="ExternalOutput").ap()

    triu_np, onesk_np, a0h_np, a0l_np, bt_np, b2p_np = _consts_fast()
    triu_c = nc.inline_tensor(triu_np.astype(np.float16), "triu_c").ap()
    onesk_c = nc.inline_tensor(onesk_np.astype(np.float16), "onesk_c").ap()
    a0h_c = nc.inline_tensor(a0h_np, "a0h_c").ap()
    a0l_c = nc.inline_tensor(a0l_np, "a0l_c").ap()
    bt_c = nc.inline_tensor(bt_np, "bt_c").ap()
    b2p_c = nc.inline_tensor(b2p_np, "b2p_c").ap()

    with tile.TileContext(nc) as tc, ExitStack() as ctx:
        wp = ctx.enter_context(tc.tile_pool(name="wp", bufs=1))
        stg = ctx.enter_context(tc.tile_pool(name="stg", bufs=2))

        # ---- index windows + embedding gathers first ----
        sent_t = wp.tile([P, NW], i32)
        nc.sync.dma_start(sent_t[:], sent_w.rearrange("(b p) -> p b", p=P))
        ptag_t = wp.tile([P, NW], i32)
        nc.sync.dma_start(ptag_t[:], ptag_w.rearrange("(b p) -> p b", p=P))
        mask_t = wp.tile([P, NW], f32)
        nc.sync.dma_start(mask_t[:], mask_w.rearrange("(b p) -> p b", p=P))
        st2_t = wp.tile([1, 512], f32)
        nc.sync.dma_start(st2_t[:], st2[None, :])

        embw, embp = [], []
        for blk in range(NW):
            tw = wp.tile([P, WDIM], f16, name=f"embw{blk}")
            nc.gpsimd.indirect_dma_start(
                out=tw[:], out_offset=None, in_=we_w[:],
                in_offset=bass.IndirectOffsetOnAxis(ap=sent_t[:, blk:blk + 1], axis=0))
            embw.append(tw)
            tpos = wp.tile([P, WDIM], f16, name=f"embp{blk}")
            nc.gpsimd.indirect_dma_start(
                out=tpos[:], out_offset=None, in_=we_p[:],
                in_offset=bass.IndirectOffsetOnAxis(ap=ptag_t[:, blk:blk + 1], axis=0))
            embp.append(tpos)

        # ---- constants ----
        triu_t = wp.tile([P, P], f16)
        nc.sync.dma_start(triu_t[:], triu_c[:])
        onesk_t = wp.tile([P, P], f16)
        nc.sync.dma_start(onesk_t[:], onesk_c[:])
        a0h_t = wp.tile([1, P], f16)
        nc.sync.dma_start(a0h_t[:], a0h_c[:])
        a0l_t = wp.tile([1, P], f16)
        nc.sync.dma_start(a0l_t[:], a0l_c[:])
        bt_t = wp.tile([P, 20], f32)
        nc.sync.dma_start(bt_t[:], bt_c[:])
        b2p_t = wp.tile([P, 20], f32)
        nc.sync.dma_start(b2p_t[:], b2p_c[:])
        st2_h = wp.tile([1, 512], f16)
        nc.vector.tensor_copy(st2_h[:], st2_t[:])

        # ---- MLP weights / biases (fp16) ----
        def load_wr(name, src_ap, n_fi):
            tiles = []
            for fi in range(n_fi):
                t = wp.tile([P, HDIM], f16, name=f"{name}{fi}", tag=f"{name}{fi}")
                s = stg.tile([P, HDIM], f32, tag="wstg", name=f"{name}s{fi}",
                             bufs=4)
                nc.scalar.dma_start(s[:], src_ap[fi * P:(fi + 1) * P, :])
                nc.vector.tensor_copy(t[:], s[:])
                tiles.append(t)
            return tiles

        w0_r = load_wr("w0r", w0, 4)
        w1_r = load_wr("w1r", w1, 4)
        ws0_r = load_wr("ws0r", ws0, 5)
        ws1_r = wp.tile([P, 4], f16)
        ws1_s = stg.tile([P, 4], f32, tag="ws1s")
        nc.scalar.dma_start(ws1_s[:], ws1.rearrange("(f p) o -> p (f o)", p=P))
        nc.vector.tensor_copy(ws1_r[:], ws1_s[:])

        def load_bias(name, src_ap):
            t = wp.tile([P, 4], f32, name=f"{name}_t", tag=f"{name}_t")
            nc.scalar.dma_start(t[:], src_ap.rearrange("(f p) -> p f", p=P))
            return t

        b0_t = load_bias("b0", b0)
        b1_t = load_bias("b1", b1)
        bs0_t = load_bias("bs0", bs0)
        bs1_t = wp.tile([1, 1], f32)
        nc.scalar.dma_start(bs1_t[:], bs1[None, :])

        # ---- masked token embeddings, feature-major prefix sum, Z = csum@w0 ----
        emb_cat = []
        for blk in range(NW):
            t = wp.tile([P, EMB], f16, name=f"embc{blk}")
            nc.vector.tensor_scalar(out=t[:, 0:WDIM], in0=embp[blk][:],
                                    scalar1=mask_t[:, blk:blk + 1], scalar2=None,
                                    op0=mybir.AluOpType.mult)
            nc.vector.tensor_scalar(out=t[:, WDIM:EMB], in0=embw[blk][:],
                                    scalar1=mask_t[:, blk:blk + 1], scalar2=None,
                                    op0=mybir.AluOpType.mult)
            emb_cat.append(t)

        pp = ExitStack()
        pr_ps = pp.enter_context(tc.tile_pool(name="pr_ps", bufs=1, space="PSUM"))

        # inclusive local csum, feature-major, via triangular + block-total
        # matmuls; col 0 of the hi/lo SBUF copies is the exclusive zero.
        lcsh, lcsl = [], []
        for fc in range(4):
            lc_ps = pr_ps.tile([P, WTOK], f32, tag="lc", name=f"lc{fc}", bufs=2)
            for tb in range(NW):
                cols = lc_ps[:, tb * P:(tb + 1) * P]
                nc.tensor.matmul(cols, emb_cat[tb][:, fc * P:(fc + 1) * P],
                                 triu_t[:], start=True, stop=(tb == 0))
                for tb2 in range(tb):
                    nc.tensor.matmul(cols, emb_cat[tb2][:, fc * P:(fc + 1) * P],
                                     onesk_t[:], start=False, stop=(tb2 == tb - 1))
            h = wp.tile([P, NZ], f16, name=f"lcsh{fc}")
            nc.vector.memset(h[:, 0:1], 0.0)
            nc.vector.tensor_copy(h[:, 1:NZ], lc_ps[:])
            lo = wp.tile([P, NZ], f16, name=f"lcsl{fc}")
            nc.vector.memset(lo[:, 0:1], 0.0)
            nc.vector.tensor_tensor(out=lo[:, 1:NZ], in0=lc_ps[:], in1=h[:, 1:NZ],
                                    op=mybir.AluOpType.subtract)
            lcsh.append(h)
            lcsl.append(lo)

        ZT = []
        for mh in range(4):
            zp = pr_ps.tile([P, NZ], f32, tag="zt", name=f"zt{mh}", bufs=2)
            k = 0
            for fc in range(4):
                nc.tensor.matmul(zp[:], w0_r[fc][:, mh * P:(mh + 1) * P],
                                 lcsh[fc][:], start=(k == 0), stop=False)
                k += 1
                nc.tensor.matmul(zp[:], w0_r[fc][:, mh * P:(mh + 1) * P],
                                 lcsl[fc][:], start=False, stop=(k == 7))
                k += 1
            z = wp.tile([P, NZ], f32, name=f"ZT{mh}")
            nc.vector.tensor_copy(z[:], zp[:])
            ZT.append(z)

        # ---- span-geometry angles: u0 = A0 * start (block-independent) ----
        u0_ps = pr_ps.tile([P, 512], f32, tag="u0", bufs=1)
        nc.tensor.matmul(u0_ps[:], a0h_t[:], st2_h[:], start=True, stop=False)
        nc.tensor.matmul(u0_ps[:], a0l_t[:], st2_h[:], start=False, stop=True)
        u0 = wp.tile([P, 512], f32, name="u0")
        nc.vector.tensor_copy(u0[:], u0_ps[:])
        pp.close()

        # ---- main span loop: 10 blocks of (2 lens x 256 starts) ----
        g_sb = ctx.enter_context(tc.tile_pool(name="g_sb", bufs=2))
        m_ps = ctx.enter_context(tc.tile_pool(name="m_ps", bufs=1, space="PSUM"))
        mp = ctx.enter_context(tc.tile_pool(name="mp", bufs=1))

        scores_sb = mp.tile([1, SH], f32)

        def make_h1(b):
            ts = [g_sb.tile([P, 512], f16, tag=f"h1_{fc}", name=f"h1_{b}_{fc}",
                            bufs=2) for fc in range(4)]
            for le in range(2):
                l = 2 * b + 1 + le
                for fc in range(4):
                    d = g_sb.tile([P, 256], f32, tag="dz",
                                  name=f"dz{b}_{le}_{fc}", bufs=3)
                    nc.vector.tensor_tensor(out=d[:], in0=ZT[fc][:, l:l + 256],
                                            in1=ZT[fc][:, 0:256],
                                            op=mybir.AluOpType.subtract)
                    nc.scalar.activation(ts[fc][:, le * 256:(le + 1) * 256], d[:],
                                         AF.Relu, bias=b0_t[:, fc:fc + 1],
                                         scale=1.0 / l)
            return ts

        h1t = {0: make_h1(0)}
        for b in range(NB):
            if b + 1 < NB:
                h1t[b + 1] = make_h1(b + 1)
            cur = h1t.pop(b)

            # sin features: round u0+bias to nearest turn, residual to Sin LUT
            ft = g_sb.tile([P, 512], f16, tag="feat", name=f"feat{b}", bufs=2)
            for le in range(2):
                l = 2 * b + 1 + le
                cols = slice(le * 256, (le + 1) * 256)
                nf = g_sb.tile([P, 256], f32, tag="nf", name=f"nf{b}_{le}",
                               bufs=2)
                # nf = round(u0 + b_l) via +MAGIC/-MAGIC (b_l in turns)
                nc.vector.tensor_scalar(out=nf[:], in0=u0[:, cols],
                                        scalar1=bt_t[:, l - 1:l],
                                        scalar2=MAGIC,
                                        op0=mybir.AluOpType.add,
                                        op1=mybir.AluOpType.add)
                nc.vector.tensor_scalar_sub(nf[:], nf[:], MAGIC)
                d0 = g_sb.tile([P, 256], f32, tag="d0", name=f"d0{b}_{le}",
                               bufs=2)
                nc.vector.tensor_tensor(out=d0[:], in0=u0[:, cols], in1=nf[:],
                                        op=mybir.AluOpType.subtract)
                # sin(2pi*d0 + 2pi*b_l) == sin(2pi*frac(u0+b_l))
                nc.scalar.activation(ft[:, cols], d0[:], AF.Sin,
                                     bias=b2p_t[:, l - 1:l], scale=TWO_PI)

            # h2 = relu(w1^T h1 + b1)
            h2 = []
            for fo in range(4):
                hp = m_ps.tile([P, 512], f32, tag="hps", name=f"h2p{b}_{fo}",
                               bufs=3)
                for fi in range(4):
                    nc.tensor.matmul(hp[:], w1_r[fi][:, fo * P:(fo + 1) * P],
                                     cur[fi][:], start=(fi == 0), stop=(fi == 3))
                ot = g_sb.tile([P, 512], f16, tag=f"h2o{fo}",
                               name=f"h2o{b}_{fo}", bufs=2)
                nc.scalar.activation(ot[:], hp[:], AF.Relu,
                                     bias=b1_t[:, fo:fo + 1], scale=1.0)
                h2.append(ot)

            # s3 = relu(ws0^T [h2; feats] + bs0)
            s3in = h2 + [ft]
            s3 = []
            for fo in range(4):
                hp = m_ps.tile([P, 512], f32, tag="hps", name=f"s3p{b}_{fo}",
                               bufs=3)
                for fi in range(5):
                    nc.tensor.matmul(hp[:], ws0_r[fi][:, fo * P:(fo + 1) * P],
                                     s3in[fi][:], start=(fi == 0), stop=(fi == 4))
                ot = g_sb.tile([P, 512], f16, tag=f"s3o{fo}",
                               name=f"s3o{b}_{fo}", bufs=2)
                nc.scalar.activation(ot[:], hp[:], AF.Relu,
                                     bias=bs0_t[:, fo:fo + 1], scale=1.0)
                s3.append(ot)

            sc_ps = m_ps.tile([1, 512], f32, tag="sc", name=f"sc{b}", bufs=2)
            for fi in range(4):
                nc.tensor.matmul(sc_ps[:], ws1_r[:, fi:fi + 1], s3[fi][:],
                                 start=(fi == 0), stop=(fi == 3))
            nc.scalar.activation(scores_sb[0:1, b * 512:(b + 1) * 512], sc_ps[:],
                                 AF.Identity, bias=bs1_t[0:1, 0:1], scale=1.0)

        nc.sync.dma_start(scores[None, :], scores_sb[:])

    nc.compile()
    return nc


def _run_fast(inputs, sst, sln):
    if "nc_fast" not in _CACHE:
        _CACHE["nc_fast"] = _build_nc_fast()
    nc = _CACHE["nc_fast"]

    sentence = np.asarray(inputs["sentence"], np.int32)
    ptags = np.asarray(inputs["pos_tags"], np.int32)
    common = {
        "We_wrd": np.asarray(inputs["We_wrd"], np.float32),
        "We_pos": np.asarray(inputs["We_pos"], np.float32),
        "dan_w0": np.asarray(inputs["dan_w0"], np.float32),
        "dan_b0": np.asarray(inputs["dan_b0"], np.float32),
        "dan_w1": np.asarray(inputs["dan_w1"], np.float32),
        "dan_b1": np.asarray(inputs["dan_b1"], np.float32),
        "ws_w0": np.asarray(inputs["ws_w0"], np.float32),
        "ws_b0": np.asarray(inputs["ws_b0"], np.float32),
        "ws_w1": np.asarray(inputs["ws_w1"], np.float32),
        "ws_b1": np.asarray(inputs["ws_b1"], np.float32),
    }
    in_maps = []
    for c in range(N_CORES):
        w = 256 * c
        tok = np.arange(w, w + WTOK)
        valid = tok < N_TOKENS
        tokc = np.minimum(tok, N_TOKENS - 1)
        m = dict(common)
        m["sent_w"] = sentence[tokc].copy()
        m["ptag_w"] = ptags[tokc].copy()
        m["mask_w"] = valid.astype(np.float32)
        st = (w + np.arange(256)).astype(np.float32)
        m["starts2"] = np.concatenate([st, st])
        in_maps.append(m)

    res = run_bass_kernel_spmd(nc, in_maps, core_ids=list(range(N_CORES)))
    grid = np.concatenate([res.results[c]["scores"] for c in range(N_CORES)])
    core = sst // 256
    pos = core * SH + (sln - 1) * 256 + (sst - core * 256)
    return res, grid[pos].astype(np.float32)


# ---------------------------------------------------------------------------
# slow fallback path (original gather-based kernel; correct for any spans)
# ---------------------------------------------------------------------------

f32r = mybir.dt.float32r


def _consts_slow():
    triu = np.triu(np.ones((P, P), np.float32))           # [k,m]=1 if k<=m
    tril = np.triu(np.ones((P, P), np.float32))           # [k,t]=1 if k<=t (cumcount rhs)
    scan = np.triu(np.ones((NTB, NTB), np.float32), 1)    # strict upper
    ident = np.eye(P, dtype=np.float32)
    iota_tags = np.tile(np.arange(N_TAGS, dtype=np.float32)[None, :], (P, 1))
    freq = np.exp(-np.log(10000.0) * (2.0 * np.arange(16) / POS_DIM)).astype(np.float64)
    c01 = [(1.0, 0.0), (1.0, 1.0), (0.0, 1.0), (1.0, 0.5)]
    b4 = np.zeros((4, P), np.float64)
    for m in range(P):
        g, j = m // 32, m % 32
        jj = j if j < 16 else j - 16
        fu = freq[jj] / (2.0 * np.pi)
        b4[0, m] = fu * c01[g][0]
        b4[1, m] = fu * c01[g][1]
        b4[2, m] = fu * c01[g][0]
        b4[3, m] = 0.0 if j < 16 else 0.25
    b4h = b4.astype(np.float16)
    b4l = (b4 - b4h.astype(np.float64)).astype(np.float16)
    return triu, tril, scan, ident, iota_tags, b4h, b4l


def _build_nc_slow():
    nc = bacc.Bacc("TRN2", target_bir_lowering=False, debug=False,
                   num_devices=N_CORES)

    def inp(name, shape, dt):
        return nc.dram_tensor(name, shape, dt, kind="ExternalInput").ap()

    sent = inp("sentence", [N_TOKENS], i32)
    ptag = inp("pos_tags", [N_TOKENS], i32)
    sst = inp("sst", [SH], i32)
    sln = inp("sln", [SH], i32)
    we_w = inp("We_wrd", [VOCAB, WDIM], f32)
    we_p = inp("We_pos", [N_TAGS, WDIM], f32)
    w0 = inp("dan_w0", [EMB, HDIM], f32)
    b0 = inp("dan_b0", [HDIM], f32)
    w1 = inp("dan_w1", [HDIM, HDIM], f32)
    b1 = inp("dan_b1", [HDIM], f32)
    ws0 = inp("ws_w0", [HDIM + FEAT, HDIM], f32)
    bs0 = inp("ws_b0", [HDIM], f32)
    ws1 = inp("ws_w1", [HDIM, 1], f32)
    bs1 = inp("ws_b1", [1], f32)

    scores = nc.dram_tensor("scores", [SH], f32, kind="ExternalOutput").ap()
    csum_d = nc.dram_tensor("csum_d", [N_TOKENS + 1, EMB], f32,
                            kind="Internal").ap()
    carry_hd = nc.dram_tensor("carry_hd", [NTB, EMB], f16, kind="Internal").ap()
    carry_ld = nc.dram_tensor("carry_ld", [NTB, EMB], f16, kind="Internal").ap()
    totals_d = nc.dram_tensor("totals_d", [NTB, EMB], f32, kind="Internal").ap()
    warm_d = nc.dram_tensor("warm_d", [P, P], f32, kind="Internal").ap()

    triu_np, tril_np, scan_np, ident_np, iota_np, b4h_np, b4l_np = _consts_slow()
    triu_c = nc.inline_tensor(triu_np.astype(np.float16), "triu_c").ap()
    tril_c = nc.inline_tensor(tril_np.astype(np.float16), "tril_c").ap()
    scan_c = nc.inline_tensor(scan_np.astype(np.float16), "scan_c").ap()
    ident_c = nc.inline_tensor(ident_np, "ident_c").ap()
    iota_c = nc.inline_tensor(iota_np, "iota_c").ap()
    b4h_c = nc.inline_tensor(b4h_np, "b4h_c").ap()
    b4l_c = nc.inline_tensor(b4l_np, "b4l_c").ap()
    ones2_c = nc.inline_tensor(np.ones((2, P), np.float16), "ones2_c").ap()
    ones512h_c = nc.inline_tensor(np.ones((1, 512), np.float16), "ones512h_c").ap()

    with tile.TileContext(nc) as tc, ExitStack() as ctx:
        wp = ctx.enter_context(tc.tile_pool(name="wp", bufs=1))
        stg = ctx.enter_context(tc.tile_pool(name="stg", bufs=2))

        sent_i = wp.tile([P, NTB], i32)
        nc.sync.dma_start(sent_i[:], sent.rearrange("(b p) -> p b", p=P))
        ptag_i = wp.tile([P, NTB], i32)
        nc.sync.dma_start(ptag_i[:], ptag.rearrange("(b p) -> p b", p=P))
        sst_t = wp.tile([P, 40], i32)
        nc.sync.dma_start(sst_t[:], sst.rearrange("(j p) -> p j", p=P))
        sln_t = wp.tile([P, 40], i32)
        nc.sync.dma_start(sln_t[:], sln.rearrange("(j p) -> p j", p=P))

        ap_ctx = ExitStack()
        ap_sb = ap_ctx.enter_context(tc.tile_pool(name="ap_sb", bufs=1))
        ap_ps = ap_ctx.enter_context(tc.tile_pool(name="ap_ps", bufs=1, space="PSUM"))
        ap_out = ap_ctx.enter_context(tc.tile_pool(name="ap_out", bufs=3))

        emb_w = ap_sb.tile([P, NTB * WDIM], f16)
        for blk in range(NTB):
            nc.gpsimd.indirect_dma_start(
                out=emb_w[:, blk * WDIM:(blk + 1) * WDIM],
                out_offset=None, in_=we_w[:],
                in_offset=bass.IndirectOffsetOnAxis(ap=sent_i[:, blk:blk + 1], axis=0))

        triu_t = wp.tile([P, P], f16)
        nc.sync.dma_start(triu_t[:], triu_c[:])
        tril_h = wp.tile([P, P], f16)
        nc.sync.dma_start(tril_h[:], tril_c[:])
        scan_t = wp.tile([NTB, NTB], f16)
        nc.sync.dma_start(scan_t[:], scan_c[:])
        b4h_t = wp.tile([4, P], f16)
        nc.sync.dma_start(b4h_t[:], b4h_c[:])
        b4l_t = wp.tile([4, P], f16)
        nc.sync.dma_start(b4l_t[:], b4l_c[:])
        iota_t = wp.tile([P, N_TAGS], f32)
        nc.sync.dma_start(iota_t[:], iota_c[:])
        wep_t = wp.tile([N_TAGS, WDIM], f32)
        nc.sync.dma_start(wep_t[:], we_p[:])
        wep_hi = wp.tile([N_TAGS, WDIM], f16)
        nc.vector.tensor_copy(wep_hi[:], wep_t[:])
        idstg = stg.tile([P, P], f32, tag="idstg")
        nc.sync.dma_start(idstg[:], ident_c[:])
        ident_h = wp.tile([P, P], f16)
        nc.vector.tensor_copy(ident_h[:], idstg[:])
        ones2_h = wp.tile([2, P], f16)
        nc.sync.dma_start(ones2_h[:], ones2_c[:])

        send_t = wp.tile([P, 40], i32)
        nc.vector.tensor_tensor(out=send_t[:], in0=sst_t[:], in1=sln_t[:],
                                op=mybir.AluOpType.add)
        nc.vector.tensor_scalar_min(send_t[:], send_t[:], N_TOKENS)
        lenf_t = wp.tile([P, 40], f32)
        nc.vector.tensor_copy(lenf_t[:], sln_t[:])
        recip_t = wp.tile([P, 40], f32)
        nc.vector.reciprocal(recip_t[:], lenf_t[:])
        ptag_f = wp.tile([P, NTB], f32)
        nc.vector.tensor_copy(ptag_f[:], ptag_i[:])

        emb_big = ap_sb.tile([P, NTB * EMB], f32)
        for blk in range(NTB):
            oh = ap_sb.tile([P, N_TAGS], f16, tag="oh", name=f"oh{blk}", bufs=2)
            nc.vector.tensor_scalar(out=oh[:], in0=iota_t[:],
                                    scalar1=ptag_f[:, blk:blk + 1], scalar2=None,
                                    op0=mybir.AluOpType.is_equal)
            ccT_ps = ap_ps.tile([N_TAGS, P], f32, tag="ccT", name=f"ccT{blk}", bufs=2)
            nc.tensor.matmul(ccT_ps[:], oh[:], tril_h[:], start=True, stop=True)
            ccT = ap_sb.tile([N_TAGS, P], f16, tag="ccTs", name=f"ccTs{blk}", bufs=3)
            nc.vector.tensor_copy(ccT[:], ccT_ps[:])
            tp = ap_ps.tile([P, EMB], f32, tag="tri_ps", name=f"tp{blk}", bufs=2)
            nc.tensor.matmul(tp[:, 0:WDIM], ccT[:], wep_hi[:],
                             start=True, stop=True)
            nc.tensor.matmul(tp[:, WDIM:EMB], triu_t[:],
                             emb_w[:, blk * WDIM:(blk + 1) * WDIM],
                             start=True, stop=True)
            nc.vector.tensor_copy(emb_big[:, blk * EMB:(blk + 1) * EMB], tp[:])
        bcsum = emb_big

        nc.sync.dma_start(totals_d.rearrange("b f -> (b f)")[None, :],
                          bcsum[P - 1:P, :])
        totals = ap_sb.tile([NTB, EMB], f32)
        nc.sync.dma_start(totals[:], totals_d[:])
        tot_hi = ap_sb.tile([NTB, EMB], f16)
        nc.vector.tensor_copy(tot_hi[:], totals[:])
        tot_lo = ap_sb.tile([NTB, EMB], f16)
        nc.vector.tensor_tensor(out=tot_lo[:], in0=totals[:], in1=tot_hi[:],
                                op=mybir.AluOpType.subtract)
        carry_ps = ap_ps.tile([NTB, EMB], f32, tag="carry_ps", bufs=1)
        nc.tensor.matmul(carry_ps[:], scan_t[:], tot_hi[:], start=True, stop=False)
        nc.tensor.matmul(carry_ps[:], scan_t[:], tot_lo[:], start=False, stop=True)
        carry_hi = ap_sb.tile([NTB, EMB], f16)
        nc.vector.tensor_copy(carry_hi[:], carry_ps[:])
        carry_lo = ap_sb.tile([NTB, EMB], f16)
        nc.vector.tensor_tensor(out=carry_lo[:], in0=carry_ps[:],
                                in1=carry_hi[:], op=mybir.AluOpType.subtract)
        nc.sync.dma_start(carry_hd[:], carry_hi[:])
        nc.scalar.dma_start(carry_ld[:], carry_lo[:])
        carry_2 = ap_sb.tile([2, NTB * EMB], f16)
        nc.sync.dma_start(carry_2[0:1, :], carry_hd.rearrange("b f -> (b f)")[None, :])
        nc.scalar.dma_start(carry_2[1:2, :], carry_ld.rearrange("b f -> (b f)")[None, :])

        zrow = ap_sb.tile([1, EMB], f32)
        nc.vector.memset(zrow[:], 0.0)
        nc.sync.dma_start(csum_d[0:1, :], zrow[:])
        for blk in range(NTB):
            bc_ps = ap_ps.tile([P, EMB], f32, tag="bc_ps", name=f"bc{blk}", bufs=3)
            nc.tensor.matmul(bc_ps[:], ones2_h[:],
                             carry_2[:, blk * EMB:(blk + 1) * EMB],
                             start=True, stop=True)
            co = ap_out.tile([P, EMB], f32, tag="co", name=f"co{blk}", bufs=5)
            nc.vector.tensor_tensor(
                out=co[:], in0=bcsum[:, blk * EMB:(blk + 1) * EMB],
                in1=bc_ps[:], op=mybir.AluOpType.add)
            nc.sync.dma_start(csum_d[1 + blk * P:1 + (blk + 1) * P, :], co[:])
        ap_ctx.close()

        def load_wr(name, src_ap, n_fi):
            tiles = []
            for fi in range(n_fi):
                t = wp.tile([P, HDIM], f16, name=f"{name}{fi}", tag=f"{name}{fi}")
                s = stg.tile([P, HDIM], f32, tag="wstg", name=f"{name}s{fi}",
                             bufs=4)
                nc.scalar.dma_start(s[:], src_ap[fi * P:(fi + 1) * P, :])
                nc.vector.tensor_copy(t[:], s[:])
                tiles.append(t)
            return tiles

        w0_r = load_wr("w0r", w0, 4)
        w1_r = load_wr("w1r", w1, 4)
        ws0_r = load_wr("ws0r", ws0, 5)
        ws1_r = wp.tile([P, 4], f16)
        ws1_s = stg.tile([P, 4], f32, tag="ws1s")
        nc.scalar.dma_start(ws1_s[:], ws1.rearrange("(f p) o -> p (f o)", p=P))
        nc.vector.tensor_copy(ws1_r[:], ws1_s[:])

        def load_bias(name, src_ap):
            t = wp.tile([P, 4], f32, name=f"{name}_t", tag=f"{name}_t")
            nc.scalar.dma_start(t[:], src_ap.rearrange("(f p) -> p f", p=P))
            return t

        b0_t = load_bias("b0", b0)
        b1_t = load_bias("b1", b1)
        bs0_t = load_bias("bs0", bs0)
        bs1_t = wp.tile([1, 1], f32)
        nc.scalar.dma_start(bs1_t[:], bs1[None, :])

        with tc.tile_pool(name="warm_pool", bufs=1, space="PSUM") as warm_pool:
            warm_ps = warm_pool.tile([P, P], f32, tag="warm", bufs=1)
            for wi in range(190):
                nc.tensor.matmul(warm_ps[:], ident_h[:], ident_h[:],
                                 start=(wi == 0), stop=(wi == 189))
            warm_sb = stg.tile([P, P], f32, tag="warm_sb")
            nc.vector.tensor_copy(warm_sb[:], warm_ps[:])
            nc.sync.dma_start(warm_d[:], warm_sb[:])

        mp = ctx.enter_context(tc.tile_pool(name="mp", bufs=1))
        g_sb = ctx.enter_context(tc.tile_pool(name="g_sb", bufs=2))
        m_ps = ctx.enter_context(tc.tile_pool(name="m_ps", bufs=1, space="PSUM"))

        scores_sb = mp.tile([1, SH], f32)

        for b in range(NB):
            ps_i = g_sb.tile([2, 512], i32, tag="ps_i", name=f"ps_i{b}")
            nc.sync.dma_start(ps_i[0:1, :], sst[None, b * 512:(b + 1) * 512])
            nc.sync.dma_start(ps_i[1:2, :], sln[None, b * 512:(b + 1) * 512])
            pos2f = g_sb.tile([2, 512], f32, tag="pos2f", name=f"pos2f{b}")
            nc.vector.tensor_copy(pos2f[:], ps_i[:])
            pos4 = g_sb.tile([4, 512], f16, tag="pos4", name=f"pos4{b}")
            nc.vector.tensor_copy(pos4[0:2, :], pos2f[:])
            slo = g_sb.tile([1, 512], f16, tag="slo", name=f"slo{b}")
            nc.vector.tensor_tensor(out=slo[:], in0=pos2f[0:1, :],
                                    in1=pos4[0:1, :],
                                    op=mybir.AluOpType.subtract)
            nc.sync.dma_start(pos4[2:3, :], slo[:])
            nc.sync.dma_start(pos4[3:4, :], ones512h_c[:])

            u_ps = m_ps.tile([P, 512], f32, tag="u_ps", name=f"u_ps{b}", bufs=2)
            nc.tensor.matmul(u_ps[:], b4h_t[:], pos4[:], start=True, stop=False)
            nc.tensor.matmul(u_ps[:], b4l_t[:], pos4[:], start=False, stop=True)
            nf = g_sb.tile([P, 512], f32, tag="nf", name=f"nf{b}", bufs=1)
            nc.vector.tensor_scalar_add(nf[:], u_ps[:], MAGIC)
            nc.vector.tensor_scalar_sub(nf[:], nf[:], MAGIC)
            df = g_sb.tile([P, 512], f32, tag="df", name=f"df{b}", bufs=1)
            nc.vector.tensor_tensor(out=df[:], in0=u_ps[:], in1=nf[:],
                                    op=mybir.AluOpType.subtract)
            feat_t = g_sb.tile([P, 512], f16, tag="feat", name=f"feat{b}")
            nc.scalar.activation(feat_t[:], df[:], AF.Sin, scale=TWO_PI)

            sm = []
            for j in range(4):
                c = 4 * b + j
                csE = g_sb.tile([P, EMB], f32, tag=f"csE{j}", name=f"csE{b}_{j}", bufs=4)
                nc.gpsimd.indirect_dma_start(
                    out=csE[:], out_offset=None, in_=csum_d[:],
                    in_offset=bass.IndirectOffsetOnAxis(ap=send_t[:, c:c + 1], axis=0))
                csS = g_sb.tile([P, EMB], f32, tag=f"csS{j}", name=f"csS{b}_{j}", bufs=4)
                nc.gpsimd.indirect_dma_start(
                    out=csS[:], out_offset=None, in_=csum_d[:],
                    in_offset=bass.IndirectOffsetOnAxis(ap=sst_t[:, c:c + 1], axis=0))
                sd = g_sb.tile([P, EMB], f16, tag=f"sd{j}", name=f"sd{b}_{j}")
                nc.vector.tensor_tensor(out=sd[:], in0=csE[:], in1=csS[:],
                                        op=mybir.AluOpType.subtract)
                smj = g_sb.tile([P, EMB], f16, tag=f"sm{j}", name=f"sm{b}_{j}")
                nc.vector.tensor_scalar(out=smj[:], in0=sd[:],
                                        scalar1=recip_t[:, c:c + 1], scalar2=None,
                                        op0=mybir.AluOpType.mult)
                sm.append(smj)

            a0 = []
            for f in range(4):
                aps = m_ps.tile([P, 512], f16, tag="a0ps", name=f"a0ps{b}_{f}", bufs=2)
                for j in range(4):
                    nc.tensor.matmul(
                        aps[:, j * P:(j + 1) * P],
                        sm[j][:, f * P:(f + 1) * P], ident_h[:],
                        is_transpose=True, start=True, stop=True)
                a0f = g_sb.tile([P, 512], f16, tag=f"a0_{f}", name=f"a0_{b}_{f}")
                nc.vector.tensor_copy(a0f[:], aps[:])
                a0.append(a0f)

            def layer(inputs, wts, bias_t, nfi, lname):
                outs = []
                for fo in range(4):
                    hp = m_ps.tile([P, 512], f32, tag="hps",
                                   name=f"{lname}{b}_{fo}", bufs=3)
                    for fi in range(nfi):
                        nc.tensor.matmul(hp[:],
                                         wts[fi][:, fo * P:(fo + 1) * P],
                                         inputs[fi][:],
                                         start=(fi == 0), stop=(fi == nfi - 1))
                    ot = g_sb.tile([P, 512], f16, tag=f"{lname}o{fo}",
                                   name=f"{lname}o{b}_{fo}")
                    nc.scalar.activation(ot[:], hp[:], AF.Relu,
                                         bias=bias_t[:, fo:fo + 1], scale=1.0)
                    outs.append(ot)
                return outs

            h1 = layer(a0, w0_r, b0_t, 4, "h1")
            h2 = layer(h1, w1_r, b1_t, 4, "h2")
            s3 = layer(h2 + [feat_t], ws0_r, bs0_t, 5, "s3")

            sc_ps = m_ps.tile([1, 512], f32, tag="sc_ps", name=f"sc_ps{b}", bufs=1)
            for fi in range(4):
                nc.tensor.matmul(sc_ps[:], ws1_r[:, fi:fi + 1], s3[fi][:],
                                 start=(fi == 0), stop=(fi == 3))
            nc.scalar.activation(scores_sb[0:1, b * 512:(b + 1) * 512], sc_ps[:],
                                 AF.Identity, bias=bs1_t[0:1, 0:1], scale=1.0)

        nc.sync.dma_start(scores[None, :], scores_sb[:])

    nc.compile()
    return nc


def _run_slow(inputs, sst0, sln0, n):
    if "nc_slow" not in _CACHE:
        _CACHE["nc_slow"] = _build_nc_slow()
    nc = _CACHE["nc_slow"]

    sst = np.zeros(N_CORES * SH, np.int32)
    sln = np.ones(N_CORES * SH, np.int32)
    sst[:n] = sst0
    sln[:n] = sln0

    common = {
        "sentence": np.asarray(inputs["sentence"], np.int32),
        "pos_tags": np.asarray(inputs["pos_tags"], np.int32),
        "We_wrd": np.asarray(inputs["We_wrd"], np.float32),
        "We_pos": np.asarray(inputs["We_pos"], np.float32),
        "dan_w0": np.asarray(inputs["dan_w0"], np.float32),
        "dan_b0": np.asarray(inputs["dan_b0"], np.float32),
        "dan_w1": np.asarray(inputs["dan_w1"], np.float32),
        "dan_b1": np.asarray(inputs["dan_b1"], np.float32),
        "ws_w0": np.asarray(inputs["ws_w0"], np.float32),
        "ws_b0": np.asarray(inputs["ws_b0"], np.float32),
        "ws_w1": np.asarray(inputs["ws_w1"], np.float32),
        "ws_b1": np.asarray(inputs["ws_b1"], np.float32),
    }
    in_maps = []
    for c in range(N_CORES):
        m = dict(common)
        m["sst"] = sst[c * SH:(c + 1) * SH].copy()
        m["sln"] = sln[c * SH:(c + 1) * SH].copy()
        in_maps.append(m)

    res = run_bass_kernel_spmd(nc, in_maps, core_ids=list(range(N_CORES)))
    out = np.concatenate([res.results[c]["scores"] for c in range(N_CORES)])
    return res, out[:n]


def kernel(**inputs):
    global LAST_RESULT
    sst = np.asarray(inputs["spans_start"], np.int64)
    sln = np.asarray(inputs["spans_len"], np.int64)
    n = sst.shape[0]
    fast_ok = (n > 0 and sln.min() >= 1 and sln.max() <= 20
               and sst.min() >= 0 and sst.max() < N_TOKENS)
    if fast_ok:
        res, out = _run_fast(inputs, sst, sln)
    else:
        res, out = _run_slow(inputs, sst.astype(np.int32), sln.astype(np.int32), n)
    LAST_RESULT = res
    return out
